# revision 8
# baseline (speedup 1.0000x reference)
"""Trainium2 Bass kernel for the tied-weight Critic MLP (v3).

Math (derived from the reference):
  x   = concat(inputs, actions)                  (B, 420), B = 8192
  s   = sum over 30 column-blocks of 14          (B, 14)
  y1  = s @ W1.T + b1                            (B, 512)
  h1  = relu(layernorm_512(y1))        [g1=1, beta1=0, LN over the 30x tile
                                        equals LN over one 512 block]
  y2  = h1 @ (30*W2).T + b2                      (B, 512)
  h2  = relu(layernorm_512(y2))
  V   = h2 @ (30*wV).T + bV                      (B, 1)
  out = tile(V, 30)                              (B, 30)

Sharding: pure data parallelism - batch 8192 split as 1024 rows on each of
8 NeuronCores; weights replicated. The kernel emits V (128,8) per core; the
30-column broadcast is done while unsharding on the host.

v3 layout (single-DMA input, PE agent-sum, fused-accumulate tail):
  * x is host-transposed to feature-major chunks: xt[r, c, t, b] with
    global feature row R = 105c + r = 14a + f, plus a constant all-ones
    row 105. ONE ~870KB DMA per core.
  * agent-sum on the PE: 4 selector matmuls (K=106) per tile, 4 tiles
    packed into one PSUM [128,128] via col-groups (32a offsets). Output
    IS the packed feature-major s-hat tile incl. the bias-slot ones rows
    (selector row 105 -> col 14 with weight 0.25 x 4 chunks). Replaces
    the old DVE tensor_reduce + DMA-transpose path entirely.
  * LN1 stats ride the PE via block-diagonal host matrices (as before).
  * rstd1 > 0 commutes with relu, so it scales the mm1 rhs columns; mm1
    emits h1 pre-normalized and feature-major -> mm2 needs no transposes.
  * Layer 2 analytically centered (W2c, b2c) -> LN2 needs only sumsq.
  * sumsq via ACT Square+accum_out reading y2 straight from PSUM; the
    relu*wv dot via DVE scalar_tensor_tensor from PSUM (no y2 SBUF copy).
  * relu evacuation of y1 split between ACT and DVE to balance engines.
  * One [128,8] f32 output DMA at the very end (the old per-group (B,1)
    store fragmented into 4-byte packets and cost ~10us of tail).
  * ~20 junk warmup matmuls at t0 keep the PE HAM un-throttled.
"""

import numpy as np

N_CORES = 8
B_FULL = 8192
B_CORE = B_FULL // N_CORES  # 1024
P = 128
N_TILES = B_CORE // P  # 8
GROUP = 4  # tiles per phase group (col-group packing width)
N_GROUPS = N_TILES // GROUP
N_AGENTS = 30
IN_F = 14
K1 = IN_F + 2  # 14 s rows + ones(b1) row + (-mu) row
KC = 105  # 420 / 4 selector chunks
KCX = KC + 1  # + constant ones row
HID = 512
EPS = 1e-5
N_WARMUP = 20  # junk matmuls at t0 to flip the PE HAM to 8/8
RELU_ON_ACT = (0, 2)  # tiles (within group) whose relu-evac runs on ACT

_cache = {}


def _build(bV: float):
    import concourse.tile as tile
    from concourse import bacc, mybir
    from concourse.bass import ts

    f32 = mybir.dt.float32
    bf16 = mybir.dt.bfloat16
    AF = mybir.ActivationFunctionType
    ALU = mybir.AluOpType

    nc = bacc.Bacc("TRN2")

    xt_d = nc.dram_tensor("xt", (KCX, 4 * B_CORE), bf16, kind="ExternalInput")
    ssel_d = nc.dram_tensor("ssel", (KCX, 4 * K1), bf16, kind="ExternalInput")
    w1rep_d = nc.dram_tensor("w1rep", (P, HID), bf16, kind="ExternalInput")
    statsb_d = nc.dram_tensor("statsb", (P, 260), bf16, kind="ExternalInput")
    onesbd_d = nc.dram_tensor("onesbd", (P, GROUP), f32, kind="ExternalInput")
    pbd_d = nc.dram_tensor("pbd", (GROUP, P), f32, kind="ExternalInput")
    w2sb_d = nc.dram_tensor("w2sb", (P, 4 * HID), bf16, kind="ExternalInput")
    b2wv_d = nc.dram_tensor("b2wv", (1, 2 * HID), bf16, kind="ExternalInput")
    out_d = nc.dram_tensor("out", (P, N_TILES), f32, kind="ExternalOutput")

    with tile.TileContext(nc) as tc:
        with (
            tc.tile_pool(name="singles", bufs=1) as singles,
            tc.tile_pool(name="s4p", bufs=2) as s4p,
            tc.tile_pool(name="gstat", bufs=2) as gstat,
            tc.tile_pool(name="hp", bufs=2) as hp,
            tc.tile_pool(name="junkp", bufs=2) as junkp,
            tc.tile_pool(name="ps1", bufs=2, space="PSUM") as ps1,
            tc.tile_pool(name="ps2", bufs=4, space="PSUM") as ps2,
            tc.tile_pool(name="pss4", bufs=1, space="PSUM") as pss4,
            tc.tile_pool(name="psst", bufs=1, space="PSUM") as psst,
        ):
            # ---- DMAs: x + small stats on the sync ring, big weights on
            # the scalar (ACT) HWDGE ring so the transfers run in parallel.
            xsb = singles.tile([KCX, 4 * B_CORE], bf16)
            nc.sync.dma_start(out=xsb, in_=xt_d[:, :])
            ssel = singles.tile([KCX, 4 * K1], bf16)
            nc.sync.dma_start(out=ssel, in_=ssel_d[:, :])
            statsb = singles.tile([P, 260], bf16)
            nc.sync.dma_start(out=statsb, in_=statsb_d[:, :])
            onesbd = singles.tile([P, GROUP], f32)
            nc.sync.dma_start(out=onesbd, in_=onesbd_d[:, :])
            pbd = singles.tile([GROUP, P], f32)
            nc.sync.dma_start(out=pbd, in_=pbd_d[:, :])
            b2wv = singles.tile([1, 2 * HID], bf16)
            nc.sync.dma_start(out=b2wv, in_=b2wv_d[:, :])
            w2sb = singles.tile([P, 4 * HID], bf16)
            nc.scalar.dma_start(out=w2sb, in_=w2sb_d[:, :])
            w1rep = singles.tile([P, HID], bf16)
            nc.scalar.dma_start(out=w1rep, in_=w1rep_d[:, :])

            wm14 = statsb[:, 0:GROUP]
            wmneg = statsb[:, GROUP : GROUP + P]
            g1bd = statsb[:, GROUP + P : GROUP + 2 * P]
            b2c = b2wv[:, 0:HID]
            wvr = b2wv[:, HID : 2 * HID]

            # ---- constants ----
            eps128 = singles.tile([P, 1], f32)
            nc.vector.memset(eps128, EPS)
            ones1 = singles.tile([1, P], bf16)
            nc.vector.memset(ones1, 1.0)
            osb = singles.tile([P, N_TILES], f32)

            # ---- PE warmup: junk matmuls to get the HAM to K=8/8 before
            # the real pipeline starts (they only depend on one memset).
            # The junk PSUM borrows a y1-pool slot (freed before tile 1).
            wjunk = singles.tile([64, P], bf16)
            nc.vector.memset(wjunk, 0.125)
            warmps = ps1.tile([64, P], f32, tag="y1n")
            for _ in range(N_WARMUP):
                nc.tensor.matmul(warmps, wjunk[:, 0:64], wjunk, start=True, stop=True)

            # ---- ACT table preload (sqrt set) + wv broadcast via PE ----
            acttbl = gstat.tile([P, 1], f32, tag="acttbl")
            nc.scalar.activation(acttbl, eps128, AF.Sqrt)
            wv_ps = ps2.tile([P, HID], f32, tag="y2")
            nc.tensor.matmul(wv_ps, ones1, wvr, start=True, stop=True)
            wv_bc = singles.tile([P, HID], bf16)
            nc.scalar.copy(out=wv_bc, in_=wv_ps)

            def group_body(g):
                # ---- A: agent-sum on the PE: packed s-hat [32a+k, b] ----
                # (big4 packs st4ps + ey4 into one PSUM bank)
                big4 = pss4.tile([P, 2, P], f32, tag="big4")
                st4ps = big4[:, 0, :]
                for c in range(4):
                    for a in range(GROUP):
                        t = g * GROUP + a
                        col = (c * N_TILES + t) * P
                        nc.tensor.matmul(
                            st4ps[32 * a : 32 * a + K1, :],
                            ssel[:, c * K1 : (c + 1) * K1],
                            xsb[:, col : col + P],
                            start=(c == 0),
                            stop=(c == 3),
                            tile_position=(0, 32 * a),
                        )
                st4 = s4p.tile([P, P], bf16, tag="st4")
                nc.vector.tensor_copy(st4, st4ps)

                # ---- B: LN1 stats on the PE (block-diagonal matrices) ----
                psb = psst.tile([P, 4, P], f32, tag="psb")
                mu4b = psb[:, 0, :]
                gs4 = psb[:, 1, :]
                rstdb = psb[:, 2, :]
                mu4 = psb[0:GROUP, 3, :]
                ey4 = big4[0:GROUP, 1, :]
                nc.tensor.matmul(mu4b, wmneg, st4, start=True, stop=True)
                nc.tensor.matmul(mu4, wm14, st4, start=True, stop=True)
                nc.tensor.matmul(gs4, g1bd, st4, start=True, stop=True)
                prod = gstat.tile([P, P], f32, tag="prod")
                nc.vector.tensor_mul(prod, st4, gs4)
                nc.tensor.matmul(ey4, onesbd, prod, start=True, stop=True)
                musq1 = gstat.tile([GROUP, P], f32, tag="musq1")
                nc.scalar.square(musq1, mu4)
                var1 = gstat.tile([GROUP, P], f32, tag="var1")
                nc.vector.tensor_sub(var1, ey4, musq1)
                std1 = gstat.tile([GROUP, P], f32, tag="std1")
                nc.scalar.activation(
                    std1, var1, AF.Sqrt, bias=eps128[0:GROUP, :], scale=1.0
                )
                rstd4 = gstat.tile([GROUP, P], f32, tag="rstd4")
                nc.vector.reciprocal_approx_fast(rstd4, std1)
                nc.tensor.matmul(rstdb, pbd, rstd4, start=True, stop=True)
                st4m = s4p.tile([P, P], bf16, tag="st4m")
                nc.vector.tensor_add(st4m, st4, mu4b)
                st4s = s4p.tile([P, P], bf16, tag="st4s")
                nc.vector.tensor_mul(st4s, st4m, rstdb)

                # ---- C-F: per-tile pipeline, software-pipelined so the PE
                # never waits on a relu handoff: mm1(0), mm1(1), then
                # {mm2(a) | mm1(a+2)} interleaved.
                y1ps = [None] * GROUP
                h1s = [None] * GROUP
                ssq4 = gstat.tile([P, GROUP], f32, tag="ssq4")
                vraw4 = gstat.tile([P, GROUP], f32, tag="vraw4")

                def mm1(a):
                    y1n = ps1.tile([P, HID], f32, tag="y1n")
                    for j in range(4):
                        nc.tensor.matmul(
                            y1n[:, ts(j, P)],
                            w1rep[32 * a : 32 * a + K1, ts(j, P)],
                            st4s[32 * a : 32 * a + K1, :],
                            start=True,
                            stop=True,
                            tile_position=(32 * a, 0),
                        )
                    y1ps[a] = y1n

                def relu(a):
                    h1n = hp.tile([P, HID], bf16, tag="h1n")
                    if a in RELU_ON_ACT:
                        nc.scalar.activation(h1n, y1ps[a], AF.Relu)
                    else:
                        nc.vector.tensor_scalar(
                            out=h1n,
                            in0=y1ps[a],
                            scalar1=0.0,
                            scalar2=None,
                            op0=ALU.max,
                        )
                    h1s[a] = h1n

                def mm2_tail(a):
                    y2 = ps2.tile([P, HID], f32, tag="y2")
                    nc.tensor.matmul(y2, ones1, b2c, start=True, stop=False)
                    for j in range(4):
                        nc.tensor.matmul(
                            y2,
                            h1s[a][:, ts(j, P)],
                            w2sb[:, j * HID : j * HID + HID],
                            start=False,
                            stop=(j == 3),
                        )
                    sqj = junkp.tile([P, HID], bf16, tag="sqj")
                    nc.scalar.activation(
                        sqj, y2, AF.Square, accum_out=ssq4[:, a : a + 1]
                    )
                    vj = junkp.tile([P, HID], bf16, tag="vj")
                    nc.vector.scalar_tensor_tensor(
                        out=vj,
                        in0=y2,
                        scalar=0.0,
                        in1=wv_bc,
                        op0=ALU.max,
                        op1=ALU.mult,
                        accum_out=vraw4[:, a : a + 1],
                    )

                mm1(0)
                relu(0)
                mm1(1)
                relu(1)
                for a in range(GROUP):
                    if a + 2 < GROUP:
                        mm1(a + 2)
                        relu(a + 2)
                    mm2_tail(a)

                # ---- G: v = vraw / sqrt(ssq/512 + eps) + bV -> osb ----
                std4 = gstat.tile([P, GROUP], f32, tag="std4")
                nc.scalar.activation(
                    std4, ssq4, AF.Sqrt, bias=eps128, scale=1.0 / HID
                )
                rstd4v = gstat.tile([P, GROUP], f32, tag="rstd4v")
                nc.vector.reciprocal_approx_fast(rstd4v, std4)
                v4 = gstat.tile([P, GROUP], f32, tag="v4")
                nc.vector.tensor_mul(v4, vraw4, rstd4v)
                nc.vector.tensor_scalar(
                    out=osb[:, g * GROUP : (g + 1) * GROUP],
                    in0=v4,
                    scalar1=float(bV),
                    scalar2=None,
                    op0=ALU.add,
                )

            for g in range(N_GROUPS):
                group_body(g)

            nc.sync.dma_start(out=out_d[:, :], in_=osb)

    nc.compile()
    return nc


def _prep(inputs):
    import ml_dtypes

    bf = ml_dtypes.bfloat16

    xin = np.concatenate(
        [
            np.asarray(inputs["inputs"], np.float32),
            np.asarray(inputs["actions"], np.float32),
        ],
        axis=1,
    )  # (8192, 420), column R = 14a + f
    w1 = np.asarray(inputs["w1"], np.float32)  # (512, 14)
    b1 = np.asarray(inputs["b1"], np.float32)  # (512,)
    w2 = np.asarray(inputs["w2"], np.float32)  # (512, 512)
    b2 = np.asarray(inputs["b2"], np.float32)
    wV = np.asarray(inputs["wV"], np.float32)
    bV = float(np.asarray(inputs["bV"], np.float32).reshape(-1)[0])

    # LN affine params are identity in this model; the kernel folds them away.
    for k, want in (("g1", 1.0), ("g2", 1.0), ("beta1", 0.0), ("beta2", 0.0)):
        if k in inputs:
            assert np.allclose(np.asarray(inputs[k]), want), f"{k} must be {want}"

    # x feature-major, selector-chunk layout + constant ones row:
    #   xt[r, c*1024 + t*128 + b] = x[core*1024 + t*128 + b, 105c + r]
    xT = np.ascontiguousarray(xin.T).astype(bf)  # (420, 8192)
    # selector: S[r, c*16 + f'] = 1 iff (105c + r) % 14 == f'; the ones row
    # feeds the bias slot (col 14) with weight 1/4 per chunk.
    ssel = np.zeros((KCX, 4, K1), np.float32)
    for c in range(4):
        for r in range(KC):
            ssel[r, c, (KC * c + r) % IN_F] = 1.0
        ssel[KC, c, IN_F] = 0.25

    what = np.concatenate([w1, b1[:, None]], axis=1)  # (512, 15)
    wm1 = what.mean(axis=0)  # (15,)
    G1 = (what.T @ what) / HID  # (15, 15)

    # lhsT for mm1: rows 0..13 = W1.T, row 14 = b1, row 15 = ones (-mu slot);
    # replicated at partition offsets 0/32/64/96.
    w1p = np.concatenate([what.T, np.ones((1, HID), np.float32)], axis=0)  # (16,512)
    w1rep = np.zeros((P, HID), np.float32)
    statsb = np.zeros((P, 260), np.float32)  # [wm14 | wmneg | g1bd]
    onesbd = np.zeros((P, GROUP), np.float32)
    pbd = np.zeros((GROUP, P), np.float32)
    for a in range(GROUP):
        o = 32 * a
        w1rep[o : o + K1, :] = w1p
        statsb[o : o + IN_F + 1, a] = wm1  # wm14
        statsb[o : o + IN_F + 1, GROUP + o + IN_F + 1] = -wm1  # wmneg
        statsb[o : o + IN_F + 1, GROUP + P + o : GROUP + P + o + IN_F + 1] = G1
        onesbd[o : o + IN_F + 1, a] = 1.0
        pbd[a, o : o + K1] = 1.0

    # layer-2 analytic centering: y2c = h1 @ W2c + b2c has zero g-mean
    w2t = (N_AGENTS * w2).T.astype(np.float32)  # (512f, 512g)
    w2c = w2t - w2t.mean(axis=1, keepdims=True)
    b2c = (b2 - b2.mean())[None, :]
    # w2sb[p, c*512 + n] = w2c[128c + p, n]
    w2sb = np.ascontiguousarray(
        w2c.reshape(4, P, HID).transpose(1, 0, 2).reshape(P, 4 * HID)
    )
    b2wv = np.concatenate(
        [b2c, N_AGENTS * wV.reshape(1, -1)], axis=1
    )  # (1, 1024)

    common = {
        "ssel": np.ascontiguousarray(ssel.reshape(KCX, 4 * K1)).astype(bf),
        "w1rep": w1rep.astype(bf),
        "statsb": statsb.astype(bf),
        "onesbd": onesbd,
        "pbd": pbd,
        "w2sb": w2sb.astype(bf),
        "b2wv": np.ascontiguousarray(b2wv).astype(bf),
    }
    in_maps = []
    for core in range(N_CORES):
        blk = xT[:, core * B_CORE : (core + 1) * B_CORE]  # (420, 1024)
        xt = np.empty((KCX, 4 * B_CORE), dtype=bf)
        xt[:KC, :] = blk.reshape(4, KC, B_CORE).transpose(1, 0, 2).reshape(
            KC, 4 * B_CORE
        )
        xt[KC, :] = np.float32(1.0)
        in_maps.append({"xt": xt, **common})
    return in_maps, bV


def _run(inputs, trace=False):
    from concourse.bass_utils import run_bass_kernel_spmd

    in_maps, bV = _prep(inputs)
    if "nc" not in _cache:
        _cache["nc"] = _build(bV)
    res = run_bass_kernel_spmd(
        _cache["nc"], in_maps, core_ids=list(range(N_CORES)), trace=trace
    )
    # out (128, 8) per core: row p, col t -> batch row t*128 + p
    vs = [np.asarray(m["out"], np.float32).T.reshape(B_CORE, 1) for m in res.results]
    v = np.concatenate(vs, axis=0)  # (8192, 1)
    out = np.ascontiguousarray(np.tile(v, (1, N_AGENTS))).astype(np.float32)
    return out, res


def kernel(**inputs) -> np.ndarray:
    out, _ = _run(inputs, trace=False)
    return out
